# revision 22
# baseline (speedup 1.0000x reference)
"""MoE (8 experts, top-2) Trainium2 kernel.

Strategy (per spec sharding_hint): expert parallelism. The host computes the
(cheap) router — logits, softmax, top-2, renormalized combine weights — and
dispatches each token to the cores owning its two experts ("all-to-all token
dispatch by top-k expert id" done at the sharding step, since kernel() holds
the full inputs host-side). Core e runs the expert-e FFN over its gathered
tokens, capacity-padded so all 8 cores run one SPMD program:

    Y = W2[e]^T @ gelu(W1[e]^T @ XT + b1[e])        (feature-major layouts)

Both weight matrices stay fully resident in SBUF (bf16, 128KB/partition), so
the gelu intermediate h never round-trips through DRAM: tokens stream in
512-wide tiles (512 = fp32 PSUM bank limit, and wide enough to amortize the
serialized LDWEIGHTS), stage 1 produces all 32 h chunks of a tile into SBUF,
and stage 2 consumes them ob-major with one PSUM accumulation group open per
rotating bank (4 banks stage 1 + 4 banks stage 2).  All matmuls are bf16 at
the full PE rate (1 row/cycle).  A post-pass batches the tile framework's
per-matmul PE semaphore increments onto group stops (sem-register writes
serialize on HW).  The host then scatter-adds (Y + b2[e]) * combine back
into the output.
"""

import os
import sys

import numpy as np

for _p in ("/opt/trn_rl_repo", "/root/.axon_site/_ro/trn_rl_repo"):
    if os.path.isdir(_p) and _p not in sys.path:
        sys.path.insert(0, _p)

NUM_EXPERTS = 8
TOP_K = 2
B, S, H, I = 4, 4096, 1024, 4096
T = B * S
P = 128
NT = 512           # token tile (moving dim; 512 fully hides LDWEIGHTS)
C_DEFAULT = 4352   # capacity per expert (seed-0 max count 4302), mult of 256

KH = H // P        # 8 contraction chunks for stage 1
KI = I // P        # 32 i-chunks (stage-1 outputs / stage-2 contraction)
OB = H // P        # 8 output row-blocks

_built = {}        # (C, reps) -> nc


def _batch_pe_sem_incs(nc):
    """Rescale the PE tick clock from per-matmul to per-accumulation-group
    (sem-register writes serialize at ~26ns each on HW, and every matmul
    carries one under the tile framework's tick-clock scheme).

    Non-stop matmuls lose their sem-inc; each stop matmul keeps a single
    +1, so the clock counts *stops*.  Every wait on that semaphore (any
    engine) is remapped: a wait for matmul-tick v becomes a wait for the
    first stop whose cumulative matmul count reaches v — strictly later,
    so cross-engine ordering is preserved.  In-group matmuls carry no
    waits themselves, so no wait-cycle can form."""
    import bisect
    import concourse.mybir as mybir

    for f in nc.m.functions:
        # pass 1: per tick-sem, cumulative matmul counts at each stop;
        # strip non-stop incs, keep stop incs at +1
        stops_cum = {}   # sem id -> list of cumulative mm counts at stops
        cum = {}         # sem id -> running mm count
        for blk in f.blocks:
            for inst in blk.instructions:
                if type(inst).__name__ != "InstMatmult":
                    continue
                si = inst.sync_info
                ups = list(si.on_update) if si and si.on_update else []
                incs = [u for u in ups
                        if u.sync_type == "semaphore"
                        and u.update_mode == "sem-inc"]
                if not incs:
                    continue
                assert len(incs) == 1 and incs[0].update_value == 1, incs
                u = incs[0]
                cum[u.id] = cum.get(u.id, 0) + 1
                if inst.stop_tensor_calc:
                    stops_cum.setdefault(u.id, []).append(cum[u.id])
                else:
                    inst.sync_info = mybir.SyncInfo(
                        on_wait=list(si.on_wait) if si else [],
                        on_update=[x for x in ups if x is not u],
                    )
        if not stops_cum:
            continue
        for sid, cnt in cum.items():
            assert stops_cum.get(sid) and stops_cum[sid][-1] == cnt, (
                "matmul stream must end on a stop"
            )
        # pass 2: remap every wait on those sems to stop ordinals
        for blk in f.blocks:
            for inst in blk.instructions:
                si = inst.sync_info
                if not si or not si.on_wait:
                    continue
                changed = False
                new_waits = []
                for w in si.on_wait:
                    if w.sync_type == "semaphore" and w.id in stops_cum:
                        assert w.wait_mode == "sem-ge-imm", w
                    if (w.sync_type == "semaphore" and w.id in stops_cum
                            and w.wait_mode == "sem-ge-imm"):
                        v = bisect.bisect_left(stops_cum[w.id],
                                               w.wait_value) + 1
                        new_waits.append(mybir.SyncWait(
                            sync_type="semaphore",
                            id=w.id,
                            ant_name=w.ant_name,
                            wait_mode="sem-ge-imm",
                            wait_value=v,
                            wait_reg=None,
                        ))
                        changed = True
                    else:
                        new_waits.append(w)
                if changed:
                    inst.sync_info = mybir.SyncInfo(
                        on_wait=new_waits,
                        on_update=list(si.on_update or []),
                    )


def _token_tiles(C):
    """Split C into tiles of 512 plus at most one trailing 256."""
    assert C % 256 == 0
    tiles, off = [], 0
    while C - off >= 512:
        tiles.append((off, 512))
        off += 512
    if C - off:
        tiles.append((off, C - off))
        off = C
    return tiles


def _build(C, reps=1, act="Gelu"):
    import concourse.bacc as bacc
    import concourse.mybir as mybir
    import concourse.tile as tile
    from concourse._compat import get_trn_type

    f32 = mybir.dt.float32
    bf16 = mybir.dt.bfloat16
    GELU = getattr(mybir.ActivationFunctionType, act)

    tiles = _token_tiles(C)

    nc = bacc.Bacc(
        get_trn_type() or "TRN2",
        target_bir_lowering=False,
        debug=False,
        enable_asserts=False,
    )
    xt = nc.dram_tensor("xt", [H, C], bf16, kind="ExternalInput").ap()
    w1 = nc.dram_tensor("w1", [H, I], bf16, kind="ExternalInput").ap()
    b1 = nc.dram_tensor("b1", [I], f32, kind="ExternalInput").ap()
    w2 = nc.dram_tensor("w2", [I, H], bf16, kind="ExternalInput").ap()
    ya = nc.dram_tensor("ya", [H, C], bf16, kind="ExternalOutput").ap()

    IQ = I // 4        # w1 load staging chunk

    with tile.TileContext(nc) as tc:
        with (
            tc.tile_pool(name="bias", bufs=1) as bpool,
            tc.tile_pool(name="wp", bufs=2) as wp,
            tc.tile_pool(name="xp", bufs=3) as xp,
            tc.tile_pool(name="hp", bufs=KI) as hp,
            tc.tile_pool(name="yp", bufs=8) as yp,
            tc.tile_pool(name="ps2", bufs=4, space="PSUM") as ps2,
            tc.tile_pool(name="ps1", bufs=4, space="PSUM") as ps1,
        ):
            b1sb = bpool.tile([P, KI], f32)
            nc.sync.dma_start(b1sb[:], b1.rearrange("(ib p) -> p ib", p=P))

            w1r = w1.rearrange("(ko p) i -> p ko i", p=P)
            w2r = w2.rearrange("(ko p) o -> p ko o", p=P)

            for rep in range(reps):
                w1sb = wp.tile([P, KH, I], bf16, tag="w", name=f"w1_{rep}")
                w2sb = wp.tile([P, KI, H], bf16, tag="w", name=f"w2_{rep}")

                def _load_x(t, toff, tsz):
                    xst = xp.tile([P, KH, tsz], bf16, tag="x",
                                  name=f"x_{rep}_{t}")
                    nc.sync.dma_start(
                        xst[:],
                        xt[:, toff:toff + tsz].rearrange(
                            "(ko p) n -> p ko n", p=P),
                    )
                    return xst

                # staged loads: w1 quarter 0, first x tile, then the rest —
                # so PE starts after ~3MB of DMA, not ~17MB
                nc.sync.dma_start(w1sb[:, :, 0:IQ], w1r[:, :, 0:IQ])
                x0 = _load_x(0, tiles[0][0], tiles[0][1])
                nc.sync.dma_start(w2sb[:, 0:KI // 4], w2r[:, 0:KI // 4])
                for q in range(1, 4):
                    nc.sync.dma_start(
                        w1sb[:, :, q * IQ:(q + 1) * IQ],
                        w1r[:, :, q * IQ:(q + 1) * IQ],
                    )
                    nc.sync.dma_start(
                        w2sb[:, q * (KI // 4):(q + 1) * (KI // 4)],
                        w2r[:, q * (KI // 4):(q + 1) * (KI // 4)],
                    )

                for t, (toff, tsz) in enumerate(tiles):
                    xst = x0 if t == 0 else _load_x(t, toff, tsz)
                    # stage 1: h[ib] = gelu(W1^T x + b1) for all 32 i-chunks,
                    # kept in SBUF for the whole token tile
                    hts = []
                    for ib in range(KI):
                        ps = ps1.tile([P, tsz], f32, tag="s1",
                                      name=f"s1_{rep}_{t}_{ib}")
                        for k in range(KH):
                            # two 64-col-group matmuls run concurrently on
                            # disjoint subarrays; each half's LDWEIGHTS is
                            # 64 cols and overlaps the other half's stream
                            for hf in range(2):
                                lo = ib * P + hf * 64
                                nc.tensor.matmul(
                                    ps[hf * 64:(hf + 1) * 64, :],
                                    lhsT=w1sb[:, k, lo:lo + 64],
                                    rhs=xst[:, k],
                                    start=(k == 0),
                                    stop=(k == KH - 1),
                                    skip_group_check=True,
                                )
                        ht = hp.tile([P, tsz], bf16, tag="h",
                                     name=f"h_{rep}_{t}_{ib}")
                        nc.scalar.activation(
                            ht[:], ps[:], GELU, bias=b1sb[:, ib:ib + 1]
                        )
                        hts.append(ht)
                    # stage 2: ob-major so only one PSUM accumulation group
                    # is open per rotating bank
                    for ob in range(OB):
                        ps = ps2.tile([P, tsz], f32, tag="s2",
                                      name=f"s2_{rep}_{t}_{ob}")
                        for k in range(KI):
                            for hf in range(2):
                                lo = ob * P + hf * 64
                                nc.tensor.matmul(
                                    ps[hf * 64:(hf + 1) * 64, :],
                                    lhsT=w2sb[:, k, lo:lo + 64],
                                    rhs=hts[k][:],
                                    start=(k == 0),
                                    stop=(k == KI - 1),
                                    skip_group_check=True,
                                )
                        ys = yp.tile([P, tsz], bf16, tag="y",
                                     name=f"ys_{rep}_{t}_{ob}")
                        nc.vector.tensor_copy(ys[:], ps[:])
                        nc.sync.dma_start(
                            ya[ob * P:(ob + 1) * P, toff:toff + tsz],
                            ys[:],
                        )
    _batch_pe_sem_incs(nc)
    nc.finalize()
    return nc


def _routing(hidden, router_w, router_b):
    """Top-2 routing, bit-matching the jax reference on CPU."""
    import jax
    import jax.numpy as jnp

    cpu = jax.local_devices(backend="cpu")[0]
    with jax.default_device(cpu):
        logits = jnp.einsum("bsh,he->bse", jnp.asarray(hidden),
                            jnp.asarray(router_w)) + jnp.asarray(router_b)
        probs = jax.nn.softmax(logits, axis=-1)
        tkp, tki = jax.lax.top_k(probs, TOP_K)
        tkp = tkp / jnp.sum(tkp, axis=-1, keepdims=True)
        tkp_np = np.asarray(tkp).reshape(T, TOP_K)
        tki_np = np.asarray(tki).reshape(T, TOP_K)
    return tkp_np, tki_np


def _prepare(hidden_states, w1, b1, w2, b2, router_w, router_b):
    """Host-side routing + dispatch: returns (in_maps, C, aux for unshard)."""
    hidden_states = np.ascontiguousarray(hidden_states, dtype=np.float32)
    w1 = np.ascontiguousarray(w1, dtype=np.float32)
    b1 = np.ascontiguousarray(b1, dtype=np.float32)
    w2 = np.ascontiguousarray(w2, dtype=np.float32)
    b2 = np.ascontiguousarray(b2, dtype=np.float32)

    import ml_dtypes

    bf16 = ml_dtypes.bfloat16
    w1_bf = w1.astype(bf16)
    w2_bf = w2.astype(bf16)
    tkp, tki = _routing(hidden_states, router_w, router_b)
    x = hidden_states.reshape(T, H)

    idx_e, prob_e = [], []
    for e in range(NUM_EXPERTS):
        hit = tki == e                       # [T, 2] bool
        idx = np.nonzero(hit.any(axis=1))[0]
        pe = np.where(hit[idx, 0], tkp[idx, 0], tkp[idx, 1]).astype(np.float32)
        idx_e.append(idx)
        prob_e.append(pe)

    maxn = max(len(ix) for ix in idx_e)
    C = C_DEFAULT if maxn <= C_DEFAULT else ((maxn + NT - 1) // NT) * NT

    in_maps = []
    for e in range(NUM_EXPERTS):
        ix = idx_e[e]
        xt = np.zeros((H, C), dtype=bf16)
        xt[:, :len(ix)] = x[ix].T
        in_maps.append({
            "xt": xt,
            "w1": w1_bf[e],
            "b1": b1[e],
            "w2": w2_bf[e],
        })
    return in_maps, C, (idx_e, prob_e, b2)


def _unshard(res, aux):
    idx_e, prob_e, b2 = aux
    out = np.zeros((T, H), dtype=np.float32)
    for e in range(NUM_EXPERTS):
        ix = idx_e[e]
        y = res[e]["ya"][:, :len(ix)].T
        out[ix] += (y + b2[e]) * prob_e[e][:, None]
    return out.reshape(B, S, H)


def kernel(hidden_states, w1, b1, w2, b2, router_w, router_b):
    from concourse import bass_utils

    in_maps, C, aux = _prepare(
        hidden_states, w1, b1, w2, b2, router_w, router_b
    )
    if C not in _built:
        _built[C] = _build(C)
    nc = _built[C]

    res = bass_utils.run_bass_kernel_spmd(
        nc, in_maps, core_ids=list(range(NUM_EXPERTS))
    ).results
    return _unshard(res, aux)


# revision 26
# speedup vs baseline: 1.0522x; 1.0522x over previous
"""MoE (8 experts, top-2) Trainium2 kernel.

Strategy (per spec sharding_hint): expert parallelism. The host computes the
(cheap) router — logits, softmax, top-2, renormalized combine weights — and
dispatches each token to the cores owning its two experts ("all-to-all token
dispatch by top-k expert id" done at the sharding step, since kernel() holds
the full inputs host-side). Core e runs the expert-e FFN over its gathered
tokens, capacity-padded so all 8 cores run one SPMD program:

    Y = W2[e]^T @ gelu(W1[e]^T @ XT + b1[e])        (feature-major layouts)

Both weight matrices stay fully resident in SBUF (bf16, 128KB/partition), so
the gelu intermediate h never round-trips through DRAM: tokens stream in
512-wide tiles (512 = fp32 PSUM bank limit, and wide enough to amortize the
serialized LDWEIGHTS), stage 1 produces all 32 h chunks of a tile into SBUF,
and stage 2 consumes them ob-major with one PSUM accumulation group open per
rotating bank (4 banks stage 1 + 4 banks stage 2).  All matmuls are bf16 at
the full PE rate (1 row/cycle).  A post-pass batches the tile framework's
per-matmul PE semaphore increments onto group stops (sem-register writes
serialize on HW).  The host then scatter-adds (Y + b2[e]) * combine back
into the output.
"""

import os
import sys

import numpy as np

for _p in ("/opt/trn_rl_repo", "/root/.axon_site/_ro/trn_rl_repo"):
    if os.path.isdir(_p) and _p not in sys.path:
        sys.path.insert(0, _p)

NUM_EXPERTS = 8
TOP_K = 2
B, S, H, I = 4, 4096, 1024, 4096
T = B * S
P = 128
NT = 512           # token tile (moving dim; 512 amortizes LDWEIGHTS best)
C_DEFAULT = 4304   # capacity per expert (seed-0 max count 4302), mult of 16

KH = H // P        # 8 contraction chunks for stage 1
KI = I // P        # 32 i-chunks (stage-1 outputs / stage-2 contraction)
OB = H // P        # 8 output row-blocks

_built = {}        # (C, reps) -> nc


def _batch_pe_sem_incs(nc):
    """Rescale the PE tick clock from per-matmul to per-accumulation-group
    (sem-register writes serialize at ~26ns each on HW, and every matmul
    carries one under the tile framework's tick-clock scheme).

    Non-stop matmuls lose their sem-inc; each stop matmul keeps a single
    +1, so the clock counts *stops*.  Every wait on that semaphore (any
    engine) is remapped: a wait for matmul-tick v becomes a wait for the
    first stop whose cumulative matmul count reaches v — strictly later,
    so cross-engine ordering is preserved.  In-group matmuls carry no
    waits themselves, so no wait-cycle can form."""
    import bisect
    import concourse.mybir as mybir

    for f in nc.m.functions:
        # pass 1: per tick-sem, cumulative matmul counts at each stop;
        # strip non-stop incs, keep stop incs at +1
        stops_cum = {}   # sem id -> list of cumulative mm counts at stops
        cum = {}         # sem id -> running mm count
        for blk in f.blocks:
            for inst in blk.instructions:
                if type(inst).__name__ != "InstMatmult":
                    continue
                si = inst.sync_info
                ups = list(si.on_update) if si and si.on_update else []
                incs = [u for u in ups
                        if u.sync_type == "semaphore"
                        and u.update_mode == "sem-inc"]
                if not incs:
                    continue
                assert len(incs) == 1 and incs[0].update_value == 1, incs
                u = incs[0]
                cum[u.id] = cum.get(u.id, 0) + 1
                if inst.stop_tensor_calc:
                    stops_cum.setdefault(u.id, []).append(cum[u.id])
                else:
                    inst.sync_info = mybir.SyncInfo(
                        on_wait=list(si.on_wait) if si else [],
                        on_update=[x for x in ups if x is not u],
                    )
        if not stops_cum:
            continue
        for sid, cnt in cum.items():
            assert stops_cum.get(sid) and stops_cum[sid][-1] == cnt, (
                "matmul stream must end on a stop"
            )
        # pass 2: remap every wait on those sems to stop ordinals
        for blk in f.blocks:
            for inst in blk.instructions:
                si = inst.sync_info
                if not si or not si.on_wait:
                    continue
                changed = False
                new_waits = []
                for w in si.on_wait:
                    if w.sync_type == "semaphore" and w.id in stops_cum:
                        assert w.wait_mode == "sem-ge-imm", w
                    if (w.sync_type == "semaphore" and w.id in stops_cum
                            and w.wait_mode == "sem-ge-imm"):
                        v = bisect.bisect_left(stops_cum[w.id],
                                               w.wait_value) + 1
                        new_waits.append(mybir.SyncWait(
                            sync_type="semaphore",
                            id=w.id,
                            ant_name=w.ant_name,
                            wait_mode="sem-ge-imm",
                            wait_value=v,
                            wait_reg=None,
                        ))
                        changed = True
                    else:
                        new_waits.append(w)
                if changed:
                    inst.sync_info = mybir.SyncInfo(
                        on_wait=new_waits,
                        on_update=list(si.on_update or []),
                    )


def _token_tiles(C):
    """Split C into tiles of 512 plus at most one 16-granular tail."""
    assert C % 16 == 0
    tiles, off = [], 0
    while C - off >= 512:
        tiles.append((off, 512))
        off += 512
    if C - off:
        tiles.append((off, C - off))
        off = C
    return tiles


def _build(C, reps=1, act="Gelu"):
    import concourse.bacc as bacc
    import concourse.mybir as mybir
    import concourse.tile as tile
    from concourse._compat import get_trn_type

    f32 = mybir.dt.float32
    bf16 = mybir.dt.bfloat16
    GELU = getattr(mybir.ActivationFunctionType, act)

    tiles = _token_tiles(C)

    nc = bacc.Bacc(
        get_trn_type() or "TRN2",
        target_bir_lowering=False,
        debug=False,
        enable_asserts=False,
    )
    xt = nc.dram_tensor("xt", [H, C], bf16, kind="ExternalInput").ap()
    w1 = nc.dram_tensor("w1", [H, I], bf16, kind="ExternalInput").ap()
    b1 = nc.dram_tensor("b1", [I], f32, kind="ExternalInput").ap()
    w2 = nc.dram_tensor("w2", [I, H], bf16, kind="ExternalInput").ap()
    ya = nc.dram_tensor("ya", [H, C], bf16, kind="ExternalOutput").ap()

    IQ = I // 4        # w1 load staging chunk

    with tile.TileContext(nc) as tc:
        with (
            tc.tile_pool(name="bias", bufs=1) as bpool,
            tc.tile_pool(name="wp", bufs=2) as wp,
            tc.tile_pool(name="xp", bufs=3) as xp,
            tc.tile_pool(name="hp", bufs=KI) as hp,
            tc.tile_pool(name="yp", bufs=8) as yp,
            tc.tile_pool(name="ps2", bufs=4, space="PSUM") as ps2,
            tc.tile_pool(name="ps1", bufs=4, space="PSUM") as ps1,
        ):
            b1sb = bpool.tile([P, KI], f32)
            nc.sync.dma_start(b1sb[:], b1.rearrange("(ib p) -> p ib", p=P))

            w1r = w1.rearrange("(ko p) i -> p ko i", p=P)
            w2r = w2.rearrange("(ko p) o -> p ko o", p=P)

            for rep in range(reps):
                w1sb = wp.tile([P, KH, I], bf16, tag="w", name=f"w1_{rep}")
                w2sb = wp.tile([P, KI, H], bf16, tag="w", name=f"w2_{rep}")

                def _load_x(t, toff, tsz):
                    xst = xp.tile([P, KH, tsz], bf16, tag="x",
                                  name=f"x_{rep}_{t}")
                    nc.sync.dma_start(
                        xst[:],
                        xt[:, toff:toff + tsz].rearrange(
                            "(ko p) n -> p ko n", p=P),
                    )
                    return xst

                # staged loads: w1 quarter 0, first x tile, then the rest —
                # so PE starts after ~3MB of DMA, not ~17MB
                nc.sync.dma_start(w1sb[:, :, 0:IQ], w1r[:, :, 0:IQ])
                x0 = _load_x(0, tiles[0][0], tiles[0][1])
                nc.sync.dma_start(w2sb[:, 0:KI // 4], w2r[:, 0:KI // 4])
                for q in range(1, 4):
                    nc.sync.dma_start(
                        w1sb[:, :, q * IQ:(q + 1) * IQ],
                        w1r[:, :, q * IQ:(q + 1) * IQ],
                    )
                    nc.sync.dma_start(
                        w2sb[:, q * (KI // 4):(q + 1) * (KI // 4)],
                        w2r[:, q * (KI // 4):(q + 1) * (KI // 4)],
                    )

                for t, (toff, tsz) in enumerate(tiles):
                    xst = x0 if t == 0 else _load_x(t, toff, tsz)
                    # stage 1: h[ib] = gelu(W1^T x + b1) for all 32 i-chunks,
                    # kept in SBUF for the whole token tile
                    hts = []
                    for ib in range(KI):
                        ps = ps1.tile([P, tsz], f32, tag="s1",
                                      name=f"s1_{rep}_{t}_{ib}")
                        for k in range(KH):
                            nc.tensor.matmul(
                                ps[:],
                                lhsT=w1sb[:, k, ib * P:(ib + 1) * P],
                                rhs=xst[:, k],
                                start=(k == 0),
                                stop=(k == KH - 1),
                            )
                        ht = hp.tile([P, tsz], bf16, tag="h",
                                     name=f"h_{rep}_{t}_{ib}")
                        nc.scalar.activation(
                            ht[:], ps[:], GELU, bias=b1sb[:, ib:ib + 1]
                        )
                        hts.append(ht)
                    # stage 2: ob-major so only one PSUM accumulation group
                    # is open per rotating bank
                    for ob in range(OB):
                        ps = ps2.tile([P, tsz], f32, tag="s2",
                                      name=f"s2_{rep}_{t}_{ob}")
                        for k in range(KI):
                            nc.tensor.matmul(
                                ps[:],
                                lhsT=w2sb[:, k, ob * P:(ob + 1) * P],
                                rhs=hts[k][:],
                                start=(k == 0),
                                stop=(k == KI - 1),
                            )
                        ys = yp.tile([P, tsz], bf16, tag="y",
                                     name=f"ys_{rep}_{t}_{ob}")
                        nc.vector.tensor_copy(ys[:], ps[:])
                        nc.sync.dma_start(
                            ya[ob * P:(ob + 1) * P, toff:toff + tsz],
                            ys[:],
                        )
    _batch_pe_sem_incs(nc)
    nc.finalize()
    return nc


def _routing(hidden, router_w, router_b):
    """Top-2 routing, bit-matching the jax reference on CPU."""
    import jax
    import jax.numpy as jnp

    cpu = jax.local_devices(backend="cpu")[0]
    with jax.default_device(cpu):
        logits = jnp.einsum("bsh,he->bse", jnp.asarray(hidden),
                            jnp.asarray(router_w)) + jnp.asarray(router_b)
        probs = jax.nn.softmax(logits, axis=-1)
        tkp, tki = jax.lax.top_k(probs, TOP_K)
        tkp = tkp / jnp.sum(tkp, axis=-1, keepdims=True)
        tkp_np = np.asarray(tkp).reshape(T, TOP_K)
        tki_np = np.asarray(tki).reshape(T, TOP_K)
    return tkp_np, tki_np


def _prepare(hidden_states, w1, b1, w2, b2, router_w, router_b):
    """Host-side routing + dispatch: returns (in_maps, C, aux for unshard)."""
    hidden_states = np.ascontiguousarray(hidden_states, dtype=np.float32)
    w1 = np.ascontiguousarray(w1, dtype=np.float32)
    b1 = np.ascontiguousarray(b1, dtype=np.float32)
    w2 = np.ascontiguousarray(w2, dtype=np.float32)
    b2 = np.ascontiguousarray(b2, dtype=np.float32)

    import ml_dtypes

    bf16 = ml_dtypes.bfloat16
    w1_bf = w1.astype(bf16)
    w2_bf = w2.astype(bf16)
    tkp, tki = _routing(hidden_states, router_w, router_b)
    x = hidden_states.reshape(T, H)

    idx_e, prob_e = [], []
    for e in range(NUM_EXPERTS):
        hit = tki == e                       # [T, 2] bool
        idx = np.nonzero(hit.any(axis=1))[0]
        pe = np.where(hit[idx, 0], tkp[idx, 0], tkp[idx, 1]).astype(np.float32)
        idx_e.append(idx)
        prob_e.append(pe)

    maxn = max(len(ix) for ix in idx_e)
    C = max(64, ((maxn + 15) // 16) * 16)   # exact-fit capacity, 16-granular

    in_maps = []
    for e in range(NUM_EXPERTS):
        ix = idx_e[e]
        xt = np.zeros((H, C), dtype=bf16)
        xt[:, :len(ix)] = x[ix].T
        in_maps.append({
            "xt": xt,
            "w1": w1_bf[e],
            "b1": b1[e],
            "w2": w2_bf[e],
        })
    return in_maps, C, (idx_e, prob_e, b2)


def _unshard(res, aux):
    idx_e, prob_e, b2 = aux
    out = np.zeros((T, H), dtype=np.float32)
    for e in range(NUM_EXPERTS):
        ix = idx_e[e]
        y = res[e]["ya"][:, :len(ix)].T
        out[ix] += (y + b2[e]) * prob_e[e][:, None]
    return out.reshape(B, S, H)


def kernel(hidden_states, w1, b1, w2, b2, router_w, router_b):
    from concourse import bass_utils

    in_maps, C, aux = _prepare(
        hidden_states, w1, b1, w2, b2, router_w, router_b
    )
    if C not in _built:
        _built[C] = _build(C)
    nc = _built[C]

    res = bass_utils.run_bass_kernel_spmd(
        nc, in_maps, core_ids=list(range(NUM_EXPERTS))
    ).results
    return _unshard(res, aux)


# revision 27
# speedup vs baseline: 1.0891x; 1.0351x over previous
"""MoE (8 experts, top-2) Trainium2 kernel.

Strategy (per spec sharding_hint): expert parallelism. The host computes the
(cheap) router — logits, softmax, top-2, renormalized combine weights — and
dispatches each token to the cores owning its two experts ("all-to-all token
dispatch by top-k expert id" done at the sharding step, since kernel() holds
the full inputs host-side). Core e runs the expert-e FFN over its gathered
tokens, capacity-padded so all 8 cores run one SPMD program:

    Y = W2[e]^T @ gelu(W1[e]^T @ XT + b1[e])        (feature-major layouts)

Both weight matrices stay fully resident in SBUF (bf16, 128KB/partition), so
the gelu intermediate h never round-trips through DRAM: tokens stream in
512-wide tiles (512 = fp32 PSUM bank limit, and wide enough to amortize the
serialized LDWEIGHTS), stage 1 produces all 32 h chunks of a tile into SBUF,
and stage 2 consumes them ob-major with one PSUM accumulation group open per
rotating bank (4 banks stage 1 + 4 banks stage 2).  All matmuls are bf16 at
the full PE rate (1 row/cycle).  A post-pass batches the tile framework's
per-matmul PE semaphore increments onto group stops (sem-register writes
serialize on HW).  The host then scatter-adds (Y + b2[e]) * combine back
into the output.
"""

import os
import sys

import numpy as np

for _p in ("/opt/trn_rl_repo", "/root/.axon_site/_ro/trn_rl_repo"):
    if os.path.isdir(_p) and _p not in sys.path:
        sys.path.insert(0, _p)

NUM_EXPERTS = 8
TOP_K = 2
B, S, H, I = 4, 4096, 1024, 4096
T = B * S
P = 128
NT = 512           # token tile (moving dim; 512 amortizes LDWEIGHTS best)
C_DEFAULT = 4192   # seed-0 exact-fit capacity (max expert count ~4177);
                   # kernel() computes C from the actual routing counts

KH = H // P        # 8 contraction chunks for stage 1
KI = I // P        # 32 i-chunks (stage-1 outputs / stage-2 contraction)
OB = H // P        # 8 output row-blocks

_built = {}        # (C, reps) -> nc


def _batch_pe_sem_incs(nc):
    """Rescale the PE tick clock from per-matmul to per-accumulation-group
    (sem-register writes serialize at ~26ns each on HW, and every matmul
    carries one under the tile framework's tick-clock scheme).

    Non-stop matmuls lose their sem-inc; each stop matmul keeps a single
    +1, so the clock counts *stops*.  Every wait on that semaphore (any
    engine) is remapped: a wait for matmul-tick v becomes a wait for the
    first stop whose cumulative matmul count reaches v — strictly later,
    so cross-engine ordering is preserved.  In-group matmuls carry no
    waits themselves, so no wait-cycle can form."""
    import bisect
    import concourse.mybir as mybir

    for f in nc.m.functions:
        # pass 1: per tick-sem, cumulative matmul counts at each stop;
        # strip non-stop incs, keep stop incs at +1
        stops_cum = {}   # sem id -> list of cumulative mm counts at stops
        cum = {}         # sem id -> running mm count
        for blk in f.blocks:
            for inst in blk.instructions:
                if type(inst).__name__ != "InstMatmult":
                    continue
                si = inst.sync_info
                ups = list(si.on_update) if si and si.on_update else []
                incs = [u for u in ups
                        if u.sync_type == "semaphore"
                        and u.update_mode == "sem-inc"]
                if not incs:
                    continue
                assert len(incs) == 1 and incs[0].update_value == 1, incs
                u = incs[0]
                cum[u.id] = cum.get(u.id, 0) + 1
                if inst.stop_tensor_calc:
                    stops_cum.setdefault(u.id, []).append(cum[u.id])
                else:
                    inst.sync_info = mybir.SyncInfo(
                        on_wait=list(si.on_wait) if si else [],
                        on_update=[x for x in ups if x is not u],
                    )
        if not stops_cum:
            continue
        for sid, cnt in cum.items():
            assert stops_cum.get(sid) and stops_cum[sid][-1] == cnt, (
                "matmul stream must end on a stop"
            )
        # pass 2: remap every wait on those sems to stop ordinals
        for blk in f.blocks:
            for inst in blk.instructions:
                si = inst.sync_info
                if not si or not si.on_wait:
                    continue
                changed = False
                new_waits = []
                for w in si.on_wait:
                    if w.sync_type == "semaphore" and w.id in stops_cum:
                        assert w.wait_mode == "sem-ge-imm", w
                    if (w.sync_type == "semaphore" and w.id in stops_cum
                            and w.wait_mode == "sem-ge-imm"):
                        v = bisect.bisect_left(stops_cum[w.id],
                                               w.wait_value) + 1
                        new_waits.append(mybir.SyncWait(
                            sync_type="semaphore",
                            id=w.id,
                            ant_name=w.ant_name,
                            wait_mode="sem-ge-imm",
                            wait_value=v,
                            wait_reg=None,
                        ))
                        changed = True
                    else:
                        new_waits.append(w)
                if changed:
                    inst.sync_info = mybir.SyncInfo(
                        on_wait=new_waits,
                        on_update=list(si.on_update or []),
                    )


def _token_tiles(C):
    """Split C into tiles of 512 plus at most one 16-granular tail."""
    assert C % 16 == 0
    tiles, off = [], 0
    while C - off >= 512:
        tiles.append((off, 512))
        off += 512
    if C - off:
        tiles.append((off, C - off))
        off = C
    return tiles


def _build(C, reps=1, act="Gelu"):
    import concourse.bacc as bacc
    import concourse.mybir as mybir
    import concourse.tile as tile
    from concourse._compat import get_trn_type

    f32 = mybir.dt.float32
    bf16 = mybir.dt.bfloat16
    GELU = getattr(mybir.ActivationFunctionType, act)

    tiles = _token_tiles(C)

    nc = bacc.Bacc(
        get_trn_type() or "TRN2",
        target_bir_lowering=False,
        debug=False,
        enable_asserts=False,
    )
    xt = nc.dram_tensor("xt", [H, C], bf16, kind="ExternalInput").ap()
    w1 = nc.dram_tensor("w1", [H, I], bf16, kind="ExternalInput").ap()
    b1 = nc.dram_tensor("b1", [I], f32, kind="ExternalInput").ap()
    w2 = nc.dram_tensor("w2", [I, H], bf16, kind="ExternalInput").ap()
    ya = nc.dram_tensor("ya", [H, C], bf16, kind="ExternalOutput").ap()

    IQ = I // 4        # w1 load staging chunk

    with tile.TileContext(nc) as tc:
        with (
            tc.tile_pool(name="bias", bufs=1) as bpool,
            tc.tile_pool(name="wp", bufs=2) as wp,
            tc.tile_pool(name="xp", bufs=3) as xp,
            tc.tile_pool(name="hp", bufs=KI) as hp,
            tc.tile_pool(name="yp", bufs=8) as yp,
            tc.tile_pool(name="ps2", bufs=4, space="PSUM") as ps2,
            tc.tile_pool(name="ps1", bufs=4, space="PSUM") as ps1,
        ):
            b1sb = bpool.tile([P, KI], f32)
            nc.sync.dma_start(b1sb[:], b1.rearrange("(ib p) -> p ib", p=P))

            w1r = w1.rearrange("(ko p) i -> p ko i", p=P)
            w2r = w2.rearrange("(ko p) o -> p ko o", p=P)

            for rep in range(reps):
                w1sb = wp.tile([P, KH, I], bf16, tag="w", name=f"w1_{rep}")
                w2sb = wp.tile([P, KI, H], bf16, tag="w", name=f"w2_{rep}")

                def _load_x(t, toff, tsz):
                    xst = xp.tile([P, KH, tsz], bf16, tag="x",
                                  name=f"x_{rep}_{t}")
                    nc.sync.dma_start(
                        xst[:],
                        xt[:, toff:toff + tsz].rearrange(
                            "(ko p) n -> p ko n", p=P),
                    )
                    return xst

                # staged loads: w1 quarter 0, first x tile, then the rest —
                # so PE starts after ~3MB of DMA, not ~17MB
                nc.sync.dma_start(w1sb[:, :, 0:IQ], w1r[:, :, 0:IQ])
                x0 = _load_x(0, tiles[0][0], tiles[0][1])
                nc.sync.dma_start(w2sb[:, 0:KI // 4], w2r[:, 0:KI // 4])
                for q in range(1, 4):
                    nc.sync.dma_start(
                        w1sb[:, :, q * IQ:(q + 1) * IQ],
                        w1r[:, :, q * IQ:(q + 1) * IQ],
                    )
                    nc.sync.dma_start(
                        w2sb[:, q * (KI // 4):(q + 1) * (KI // 4)],
                        w2r[:, q * (KI // 4):(q + 1) * (KI // 4)],
                    )

                for t, (toff, tsz) in enumerate(tiles):
                    xst = x0 if t == 0 else _load_x(t, toff, tsz)
                    # stage 1: h[ib] = gelu(W1^T x + b1) for all 32 i-chunks,
                    # kept in SBUF for the whole token tile
                    hts = []
                    for ib in range(KI):
                        ps = ps1.tile([P, tsz], f32, tag="s1",
                                      name=f"s1_{rep}_{t}_{ib}")
                        for k in range(KH):
                            nc.tensor.matmul(
                                ps[:],
                                lhsT=w1sb[:, k, ib * P:(ib + 1) * P],
                                rhs=xst[:, k],
                                start=(k == 0),
                                stop=(k == KH - 1),
                            )
                        ht = hp.tile([P, tsz], bf16, tag="h",
                                     name=f"h_{rep}_{t}_{ib}")
                        nc.scalar.activation(
                            ht[:], ps[:], GELU, bias=b1sb[:, ib:ib + 1]
                        )
                        hts.append(ht)
                    # stage 2: ob-major so only one PSUM accumulation group
                    # is open per rotating bank
                    for ob in range(OB):
                        ps = ps2.tile([P, tsz], f32, tag="s2",
                                      name=f"s2_{rep}_{t}_{ob}")
                        for k in range(KI):
                            nc.tensor.matmul(
                                ps[:],
                                lhsT=w2sb[:, k, ob * P:(ob + 1) * P],
                                rhs=hts[k][:],
                                start=(k == 0),
                                stop=(k == KI - 1),
                            )
                        ys = yp.tile([P, tsz], bf16, tag="y",
                                     name=f"ys_{rep}_{t}_{ob}")
                        nc.vector.tensor_copy(ys[:], ps[:])
                        nc.sync.dma_start(
                            ya[ob * P:(ob + 1) * P, toff:toff + tsz],
                            ys[:],
                        )
    _batch_pe_sem_incs(nc)
    nc.finalize()
    return nc


def _routing(hidden, router_w, router_b):
    """Top-2 routing, bit-matching the jax reference on CPU."""
    import jax
    import jax.numpy as jnp

    cpu = jax.local_devices(backend="cpu")[0]
    with jax.default_device(cpu):
        logits = jnp.einsum("bsh,he->bse", jnp.asarray(hidden),
                            jnp.asarray(router_w)) + jnp.asarray(router_b)
        probs = jax.nn.softmax(logits, axis=-1)
        tkp, tki = jax.lax.top_k(probs, TOP_K)
        tkp = tkp / jnp.sum(tkp, axis=-1, keepdims=True)
        tkp_np = np.asarray(tkp).reshape(T, TOP_K)
        tki_np = np.asarray(tki).reshape(T, TOP_K)
    return tkp_np, tki_np


def _prepare(hidden_states, w1, b1, w2, b2, router_w, router_b):
    """Host-side routing + dispatch: returns (in_maps, C, aux for unshard)."""
    hidden_states = np.ascontiguousarray(hidden_states, dtype=np.float32)
    w1 = np.ascontiguousarray(w1, dtype=np.float32)
    b1 = np.ascontiguousarray(b1, dtype=np.float32)
    w2 = np.ascontiguousarray(w2, dtype=np.float32)
    b2 = np.ascontiguousarray(b2, dtype=np.float32)

    import ml_dtypes

    bf16 = ml_dtypes.bfloat16
    w1_bf = w1.astype(bf16)
    w2_bf = w2.astype(bf16)
    tkp, tki = _routing(hidden_states, router_w, router_b)
    x = hidden_states.reshape(T, H)

    idx_e, prob_e = [], []
    for e in range(NUM_EXPERTS):
        hit = tki == e                       # [T, 2] bool
        idx = np.nonzero(hit.any(axis=1))[0]
        pe = np.where(hit[idx, 0], tkp[idx, 0], tkp[idx, 1]).astype(np.float32)
        idx_e.append(idx)
        prob_e.append(pe)

    maxn = max(len(ix) for ix in idx_e)
    C = max(64, ((maxn + 15) // 16) * 16)   # exact-fit capacity, 16-granular

    in_maps = []
    for e in range(NUM_EXPERTS):
        ix = idx_e[e]
        xt = np.zeros((H, C), dtype=bf16)
        xt[:, :len(ix)] = x[ix].T
        in_maps.append({
            "xt": xt,
            "w1": w1_bf[e],
            "b1": b1[e],
            "w2": w2_bf[e],
        })
    return in_maps, C, (idx_e, prob_e, b2)


def _unshard(res, aux):
    idx_e, prob_e, b2 = aux
    out = np.zeros((T, H), dtype=np.float32)
    for e in range(NUM_EXPERTS):
        ix = idx_e[e]
        y = res[e]["ya"][:, :len(ix)].T
        out[ix] += (y + b2[e]) * prob_e[e][:, None]
    return out.reshape(B, S, H)


def kernel(hidden_states, w1, b1, w2, b2, router_w, router_b):
    from concourse import bass_utils

    in_maps, C, aux = _prepare(
        hidden_states, w1, b1, w2, b2, router_w, router_b
    )
    if C not in _built:
        _built[C] = _build(C)
    nc = _built[C]

    res = bass_utils.run_bass_kernel_spmd(
        nc, in_maps, core_ids=list(range(NUM_EXPERTS))
    ).results
    return _unshard(res, aux)
